# revision 18
# baseline (speedup 1.0000x reference)
"""Masked dot-product attention (B=2,H=16,L=2048,D=128) on 8 trn2 NeuronCores.

Strategy:
  - Shard batch*heads: core c handles (b=0,h=2c),(0,2c+1),(1,2c),(1,2c+1) -> 4 slots.
  - Per (b,h): compute S^T[k,q] = K Q^T directly on the PE (lhsT = k-tile
    transposed to [D,k], rhs = q transposed to [D,q]) so softmax masking is a
    per-partition bias on the exp eviction, and no P-transposes are needed.
  - Only ceil(valid_len/128) key tiles are computed (the rest contribute
    exactly 0 after exp of -1e9, matching the reference's mask fill).
  - exp is fused into the PSUM->SBUF eviction on the scalar engine with
    scale = 1/sqrt(D); j pairs share one exp instruction. The last (partial)
    key tile gets a per-partition -1e9 bias.
  - O^T[d,q] += V_j^T P^T_j accumulates in PSUM (fp32); the softmax
    denominator l accumulates via all-ones [128,1] lhsT matmuls whose [1,512]
    outputs stack at partition offsets 32*qb inside a single PSUM bank.
  - l is reshaped to [q%128, q//128] layout via a DRAM bounce, reciprocal on
    the DVE, then O^T transposes back to [q,d] on the PE with the final
    eviction scaled by 1/l per partition. Hot matmuls use float32r (~12-bit
    mantissa, 4x fp32 PE throughput); accumulation stays fp32 in PSUM.
"""

import math
import os

import numpy as np

try:
    import concourse.bass as bass
except ImportError:  # pragma: no cover
    import sys

    sys.path.append("/opt/trn_rl_repo")
    import concourse.bass as bass

import concourse.mybir as mybir
import concourse.tile as tile
from concourse import bacc
from concourse.bass_utils import run_bass_kernel_spmd

B, H, L, D = 2, 16, 2048, 128
NCORES = 8
HPC = H // NCORES  # heads per core per batch
SLOTS = B * HPC  # bh slots per core
NEG = -1e9
INV_SQRT_D = 1.0 / math.sqrt(D)
F32 = mybir.dt.float32
F32R = mybir.dt.float32r
QT = L // 128  # 16 q tiles
QB = 4  # q blocks
QBW = L // QB  # 512 q per block
QTB = QT // QB  # 4 q tiles per block
EXPF = mybir.ActivationFunctionType.Exp

_cache: dict = {}


def _build(K0: int, K1: int):
    """Build+compile the per-core program for K0/K1 valid key tiles."""
    Ks = [K0, K0, K1, K1]
    KM = max(K0, K1)
    nc = bacc.Bacc("TRN2", target_bir_lowering=False, debug=False, num_devices=NCORES)
    q = nc.dram_tensor("q", [SLOTS, L, D], F32R, kind="ExternalInput")
    k = nc.dram_tensor("k", [SLOTS, KM * 128, D], F32R, kind="ExternalInput")
    v = nc.dram_tensor("v", [SLOTS, KM * 128, D], F32R, kind="ExternalInput")
    identr = nc.dram_tensor("identr", [128, 128], F32R, kind="ExternalInput")
    identf = nc.dram_tensor("identf", [128, 128], F32, kind="ExternalInput")
    onesr = nc.dram_tensor("onesr", [128, 1], F32R, kind="ExternalInput")
    onef = nc.dram_tensor("onef", [1, 1], F32, kind="ExternalInput")
    biases = nc.dram_tensor("biases", [128, SLOTS], F32, kind="ExternalInput")
    out = nc.dram_tensor("out", [SLOTS, L, D], F32, kind="ExternalOutput")

    # j pairs: all-but-last j grouped in twos, last j always alone (it takes
    # the mask bias)
    def jgroups(Kv):
        gs = []
        jj = 0
        while jj < Kv - 1:
            n = 2 if jj + 2 <= Kv - 1 else 1
            gs.append((jj, n))
            jj += n
        gs.append((Kv - 1, 1))
        return gs

    with tile.TileContext(nc) as tc:
        with (
            tc.tile_pool(name="const", bufs=1) as constp,
            tc.tile_pool(name="io", bufs=2) as iop,
            tc.tile_pool(name="work", bufs=3) as workp,
            tc.tile_pool(name="psst", bufs=2, space="PSUM") as psst,
            tc.tile_pool(name="pstr", bufs=2, space="PSUM") as pstr,
            tc.tile_pool(name="psac", bufs=1, space="PSUM") as psac,
            tc.tile_pool(name="dram", bufs=2, space="DRAM") as dramp,
        ):
            ident_r = constp.tile([128, 128], F32R)
            nc.sync.dma_start(out=ident_r, in_=identr[:, :])
            ident_f = constp.tile([128, 128], F32)
            nc.sync.dma_start(out=ident_f, in_=identf[:, :])
            ones_r = constp.tile([128, 1], F32R)
            nc.sync.dma_start(out=ones_r, in_=onesr[:, :])
            one_f = constp.tile([1, 1], F32)
            nc.sync.dma_start(out=one_f, in_=onef[:, :])
            bias_sb = constp.tile([128, SLOTS], F32)
            nc.sync.dma_start(out=bias_sb, in_=biases[:, :])

            def emit_finish(s, oT_slot, lrec):
                o_sb = workp.tile([128, QT, 128], F32, tag="o_sb", bufs=2)
                for g in range(QT // 4):
                    otr = pstr.tile([128, 4, 128], F32, tag="tr")
                    for ii in range(4):
                        nc.tensor.transpose(
                            otr[:, ii, :], oT_slot[:, g * 4 + ii, :], ident_f
                        )
                    for ii in range(4):
                        t = g * 4 + ii
                        nc.vector.tensor_scalar_mul(
                            o_sb[:, t, :], otr[:, ii, :], lrec[:, t : t + 1]
                        )
                nc.sync.dma_start(
                    out=out[s].rearrange("(b p t) d -> p b t d", p=128, t=QTB),
                    in_=o_sb.rearrange("p (b t) d -> p b t d", t=QTB),
                )

            def emit_qb_finish(s, qb, oT_slot, l_sbq):
                # tiny K=1 matmuls transpose l into per-partition layout
                ltq = pstr.tile([128, QTB], F32, tag="tr")
                for t in range(QTB):
                    nc.tensor.matmul(
                        ltq[:, t : t + 1],
                        l_sbq[:, t * 128 : (t + 1) * 128],
                        one_f[:, :],
                        start=(t == 0),
                        stop=(t == QTB - 1),
                        skip_group_check=True,
                    )
                lrecq = workp.tile([128, QTB], F32, tag="lrecq")
                nc.vector.reciprocal(lrecq, ltq)
                o_sbq = workp.tile([128, QTB, 128], F32, tag="o_sbq")
                otr = pstr.tile([128, 4, 128], F32, tag="tr")
                for ii in range(QTB):
                    nc.tensor.transpose(
                        otr[:, ii, :], oT_slot[:, qb * QTB + ii, :], ident_f
                    )
                for ii in range(QTB):
                    nc.vector.tensor_scalar_mul(
                        o_sbq[:, ii, :], otr[:, ii, :], lrecq[:, ii : ii + 1]
                    )
                nc.sync.dma_start(
                    out=out[s].rearrange("(b p t) d -> p b t d", p=128, t=QTB)[
                        :, qb, :, :
                    ],
                    in_=o_sbq,
                )

            pending = None
            pending_qb = None
            order = sorted(range(SLOTS), key=lambda x: -Ks[x])
            for idx, s in enumerate(order):
                Kv = Ks[s]
                is_last = idx == SLOTS - 1
                kn = iop.tile([128, KM, 128], F32R, tag="kn")
                nc.sync.dma_start(
                    out=kn[:, :Kv, :],
                    in_=k[s, : Kv * 128, :].rearrange("(t p) d -> p t d", p=128),
                )
                vn = iop.tile([128, KM, 128], F32R, tag="vn")
                nc.sync.dma_start(
                    out=vn[:, :Kv, :],
                    in_=v[s, : Kv * 128, :].rearrange("(t p) d -> p t d", p=128),
                )
                # k -> kT [D, k]
                kTt = iop.tile([128, KM, 128], F32R, tag="kT")
                for g in range((Kv + 3) // 4):
                    n = min(4, Kv - g * 4)
                    trp = pstr.tile([128, 4, 128], F32R, tag="tr")
                    for ii in range(n):
                        nc.tensor.transpose(trp[:, ii, :], kn[:, g * 4 + ii, :], ident_r)
                    nc.scalar.copy(kTt[:, g * 4 : g * 4 + n, :], trp[:, :n, :])

                oT_slot = workp.tile([128, QT, 128], F32, tag="oT_slot", bufs=2)
                l_slot = workp.tile([1, L], F32, tag="l_slot")

                for qb in range(QB):
                    # lazy q load + transpose for this q block
                    qn = workp.tile([128, QTB, 128], F32R, tag="qn")
                    nc.sync.dma_start(
                        out=qn,
                        in_=q[s, qb * QBW : (qb + 1) * QBW, :].rearrange(
                            "(p t) d -> p t d", p=128
                        ),
                    )
                    qTt = workp.tile([128, QTB, 128], F32R, tag="qT")
                    trp = pstr.tile([128, 4, 128], F32R, tag="tr")
                    for ii in range(QTB):
                        nc.tensor.transpose(trp[:, ii, :], qn[:, ii, :], ident_r)
                    nc.vector.tensor_copy(qTt, trp)

                    oT_ps = psac.tile([128, QBW], F32, tag="oT")
                    l_ps = psac.tile([1, QBW], F32, tag="l")
                    for (j0, npair) in jgroups(Kv):
                        st = psst.tile([128, 2, QBW], F32, tag="st")
                        for jj in range(npair):
                            nc.tensor.matmul(
                                st[:, jj, :],
                                kTt[:, j0 + jj, :],
                                qTt,
                                start=True,
                                stop=True,
                            )
                        pT = workp.tile([128, 2, QBW], F32R, tag="pT")
                        last = j0 + npair == Kv
                        nc.scalar.activation(
                            pT[:, :npair, :],
                            st[:, :npair, :],
                            EXPF,
                            bias=(bias_sb[:, s : s + 1] if last else 0.0),
                            scale=INV_SQRT_D,
                        )
                        for jj in range(npair):
                            j = j0 + jj
                            nc.tensor.matmul(
                                oT_ps,
                                vn[:, j, :],
                                pT[:, jj, :],
                                start=(j == 0),
                                stop=(j == Kv - 1),
                            )
                            nc.tensor.matmul(
                                l_ps,
                                ones_r,
                                pT[:, jj, :],
                                start=(j == 0),
                                stop=(j == Kv - 1),
                            )
                    nc.vector.tensor_copy(oT_slot[:, qb * QTB : (qb + 1) * QTB, :], oT_ps)
                    if not is_last:
                        nc.scalar.copy(l_slot[:, qb * QBW : (qb + 1) * QBW], l_ps)
                    else:
                        l_sbq = workp.tile([1, QBW], F32, tag="l_sbq")
                        nc.scalar.copy(l_sbq, l_ps)
                        if pending_qb is not None:
                            emit_qb_finish(*pending_qb)
                        pending_qb = (s, qb, oT_slot, l_sbq)
                        if qb == 0 and pending is not None:
                            emit_finish(*pending)
                            pending = None

                if not is_last:
                    lrec = workp.tile([128, QT], F32, tag="lrec")
                    # l: [1, 2048] -> DRAM bounce -> [q%128, q//128]
                    lrows = dramp.tile([1, L], F32, tag="lrows")
                    nc.sync.dma_start(out=lrows, in_=l_slot)
                    lcol = workp.tile([128, QT], F32, tag="lcol")
                    nc.sync.dma_start(
                        out=lcol,
                        in_=lrows[0, :].rearrange("(t p) -> p t", p=128),
                    )
                    nc.vector.reciprocal(lrec, lcol)
                    # defer the O^T -> O finish by one slot so its l-latency
                    # hides under the next slot's compute
                    if pending is not None:
                        emit_finish(*pending)
                    pending = (s, oT_slot, lrec)
            if pending is not None:
                emit_finish(*pending)
            if pending_qb is not None:
                emit_qb_finish(*pending_qb)
    nc.compile()
    return nc


def _get_program(K0: int, K1: int):
    key = (K0, K1)
    if key not in _cache:
        _cache[key] = _build(K0, K1)
    return _cache[key]


def _run(q, k, v, valid_lens, trace=False):
    q = np.ascontiguousarray(np.asarray(q, dtype=np.float32))
    k = np.ascontiguousarray(np.asarray(k, dtype=np.float32))
    v = np.ascontiguousarray(np.asarray(v, dtype=np.float32))
    vl = np.asarray(valid_lens).astype(np.int64)
    K0 = int(max(1, -(-vl[0] // 128)))
    K1 = int(max(1, -(-vl[1] // 128)))
    KM = max(K0, K1)
    nc = _get_program(K0, K1)

    # per-slot mask bias column: 0 for valid positions in the last key tile,
    # -1e9 beyond valid_len
    biases = np.zeros((128, SLOTS), dtype=np.float32)
    Ks = [K0, K0, K1, K1]
    bs = [0, 0, 1, 1]
    pos = np.arange(128)
    for s in range(SLOTS):
        rem = int(vl[bs[s]]) - (Ks[s] - 1) * 128
        biases[:, s] = np.where(pos < rem, 0.0, np.float32(NEG))

    identf = np.eye(128, dtype=np.float32)
    onesr = np.ones((128, 1), dtype=np.float32)

    in_maps = []
    for c in range(NCORES):
        h0, h1 = 2 * c, 2 * c + 1
        qs = np.ascontiguousarray(
            np.stack([q[0, h0], q[0, h1], q[1, h0], q[1, h1]])
        )
        ks = np.ascontiguousarray(
            np.stack(
                [
                    k[0, h0, : KM * 128],
                    k[0, h1, : KM * 128],
                    k[1, h0, : KM * 128],
                    k[1, h1, : KM * 128],
                ]
            )
        )
        vs = np.ascontiguousarray(
            np.stack(
                [
                    v[0, h0, : KM * 128],
                    v[0, h1, : KM * 128],
                    v[1, h0, : KM * 128],
                    v[1, h1, : KM * 128],
                ]
            )
        )
        in_maps.append(
            {
                "q": qs,
                "k": ks,
                "v": vs,
                "identr": identf,
                "identf": identf,
                "onesr": onesr,
                "onef": onesr[:1, :1],
                "biases": biases,
            }
        )

    res = run_bass_kernel_spmd(
        nc, in_maps, core_ids=list(range(NCORES)), trace=trace
    )

    outp = np.empty((B, H, L, D), dtype=np.float32)
    for c in range(NCORES):
        o = res.results[c]["out"]
        h0, h1 = 2 * c, 2 * c + 1
        outp[0, h0] = o[0]
        outp[0, h1] = o[1]
        outp[1, h0] = o[2]
        outp[1, h1] = o[3]
    return outp, res


def kernel(q, k, v, valid_lens):
    outp, _ = _run(q, k, v, valid_lens, trace=False)
    return outp


# revision 19
# speedup vs baseline: 1.0272x; 1.0272x over previous
"""Masked dot-product attention (B=2,H=16,L=2048,D=128) on 8 trn2 NeuronCores.

Strategy:
  - Shard batch*heads: core c handles (b=0,h=2c),(0,2c+1),(1,2c),(1,2c+1) -> 4 slots.
  - Per (b,h): compute S^T[k,q] = K Q^T directly on the PE (lhsT = k-tile
    transposed to [D,k], rhs = q transposed to [D,q]) so softmax masking is a
    per-partition bias on the exp eviction, and no P-transposes are needed.
  - Only ceil(valid_len/128) key tiles are computed (the rest contribute
    exactly 0 after exp of -1e9, matching the reference's mask fill).
  - exp is fused into the PSUM->SBUF eviction on the scalar engine with
    scale = 1/sqrt(D); j pairs share one exp instruction. The last (partial)
    key tile gets a per-partition -1e9 bias.
  - O^T[d,q] += V_j^T P^T_j accumulates in PSUM (fp32); the softmax
    denominator l accumulates via all-ones [128,1] lhsT matmuls whose [1,512]
    outputs stack at partition offsets 32*qb inside a single PSUM bank.
  - l is reshaped to [q%128, q//128] layout via a DRAM bounce, reciprocal on
    the DVE, then O^T transposes back to [q,d] on the PE with the final
    eviction scaled by 1/l per partition. Hot matmuls use float32r (~12-bit
    mantissa, 4x fp32 PE throughput); accumulation stays fp32 in PSUM.
"""

import math
import os

import numpy as np

try:
    import concourse.bass as bass
except ImportError:  # pragma: no cover
    import sys

    sys.path.append("/opt/trn_rl_repo")
    import concourse.bass as bass

import concourse.mybir as mybir
import concourse.tile as tile
from concourse import bacc
from concourse.bass_utils import run_bass_kernel_spmd

B, H, L, D = 2, 16, 2048, 128
NCORES = 8
HPC = H // NCORES  # heads per core per batch
SLOTS = B * HPC  # bh slots per core
NEG = -1e9
INV_SQRT_D = 1.0 / math.sqrt(D)
F32 = mybir.dt.float32
F32R = mybir.dt.float32r
QT = L // 128  # 16 q tiles
QB = 4  # q blocks
QBW = L // QB  # 512 q per block
QTB = QT // QB  # 4 q tiles per block
EXPF = mybir.ActivationFunctionType.Exp

_cache: dict = {}


def _build(K0: int, K1: int):
    """Build+compile the per-core program for K0/K1 valid key tiles."""
    Ks = [K0, K0, K1, K1]
    KM = max(K0, K1)
    nc = bacc.Bacc("TRN2", target_bir_lowering=False, debug=False, num_devices=NCORES)
    q = nc.dram_tensor("q", [SLOTS, L, D], F32R, kind="ExternalInput")
    k = nc.dram_tensor("k", [SLOTS, KM * 128, D], F32R, kind="ExternalInput")
    v = nc.dram_tensor("v", [SLOTS, KM * 128, D], F32R, kind="ExternalInput")
    identr = nc.dram_tensor("identr", [128, 128], F32R, kind="ExternalInput")
    identf = nc.dram_tensor("identf", [128, 128], F32, kind="ExternalInput")
    onesr = nc.dram_tensor("onesr", [128, 1], F32R, kind="ExternalInput")
    onef = nc.dram_tensor("onef", [1, 1], F32, kind="ExternalInput")
    biases = nc.dram_tensor("biases", [128, SLOTS], F32, kind="ExternalInput")
    out = nc.dram_tensor("out", [SLOTS, L, D], F32, kind="ExternalOutput")

    # j pairs: all-but-last j grouped in twos, last j always alone (it takes
    # the mask bias)
    def jgroups(Kv):
        gs = []
        jj = 0
        while jj < Kv - 1:
            n = 2 if jj + 2 <= Kv - 1 else 1
            gs.append((jj, n))
            jj += n
        gs.append((Kv - 1, 1))
        return gs

    with tile.TileContext(nc) as tc:
        with (
            tc.tile_pool(name="const", bufs=1) as constp,
            tc.tile_pool(name="io", bufs=2) as iop,
            tc.tile_pool(name="work", bufs=3) as workp,
            tc.tile_pool(name="psst", bufs=2, space="PSUM") as psst,
            tc.tile_pool(name="pstr", bufs=2, space="PSUM") as pstr,
            tc.tile_pool(name="psac", bufs=1, space="PSUM") as psac,
            tc.tile_pool(name="dram", bufs=2, space="DRAM") as dramp,
        ):
            ident_r = constp.tile([128, 128], F32R)
            nc.sync.dma_start(out=ident_r, in_=identr[:, :])

            def emit_kv_loads(s):
                Kv = Ks[s]
                kn = iop.tile([128, KM, 128], F32R, tag="kn")
                nc.sync.dma_start(
                    out=kn[:, :Kv, :],
                    in_=k[s, : Kv * 128, :].rearrange("(t p) d -> p t d", p=128),
                )
                vn = iop.tile([128, KM, 128], F32R, tag="vn")
                nc.sync.dma_start(
                    out=vn[:, :Kv, :],
                    in_=v[s, : Kv * 128, :].rearrange("(t p) d -> p t d", p=128),
                )
                return kn, vn

            order0 = sorted(range(SLOTS), key=lambda x: -Ks[x])
            preload = {order0[0]: emit_kv_loads(order0[0])}

            ident_f = constp.tile([128, 128], F32)
            nc.sync.dma_start(out=ident_f, in_=identf[:, :])
            ones_r = constp.tile([128, 1], F32R)
            nc.sync.dma_start(out=ones_r, in_=onesr[:, :])
            one_f = constp.tile([1, 1], F32)
            nc.sync.dma_start(out=one_f, in_=onef[:, :])
            bias_sb = constp.tile([128, SLOTS], F32)
            nc.sync.dma_start(out=bias_sb, in_=biases[:, :])

            def emit_finish(s, oT_slot, lrec):
                o_sb = workp.tile([128, QT, 128], F32, tag="o_sb", bufs=2)
                for g in range(QT // 4):
                    otr = pstr.tile([128, 4, 128], F32, tag="tr")
                    for ii in range(4):
                        nc.tensor.transpose(
                            otr[:, ii, :], oT_slot[:, g * 4 + ii, :], ident_f
                        )
                    for ii in range(4):
                        t = g * 4 + ii
                        nc.vector.tensor_scalar_mul(
                            o_sb[:, t, :], otr[:, ii, :], lrec[:, t : t + 1]
                        )
                nc.sync.dma_start(
                    out=out[s].rearrange("(b p t) d -> p b t d", p=128, t=QTB),
                    in_=o_sb.rearrange("p (b t) d -> p b t d", t=QTB),
                )

            def emit_qb_finish(s, qb, oT_slot, l_sbq):
                # tiny K=1 matmuls transpose l into per-partition layout
                ltq = pstr.tile([128, QTB], F32, tag="tr")
                for t in range(QTB):
                    nc.tensor.matmul(
                        ltq[:, t : t + 1],
                        l_sbq[:, t * 128 : (t + 1) * 128],
                        one_f[:, :],
                        start=(t == 0),
                        stop=(t == QTB - 1),
                        skip_group_check=True,
                    )
                lrecq = workp.tile([128, QTB], F32, tag="lrecq")
                nc.vector.reciprocal(lrecq, ltq)
                o_sbq = workp.tile([128, QTB, 128], F32, tag="o_sbq")
                otr = pstr.tile([128, 4, 128], F32, tag="tr")
                for ii in range(QTB):
                    nc.tensor.transpose(
                        otr[:, ii, :], oT_slot[:, qb * QTB + ii, :], ident_f
                    )
                for ii in range(QTB):
                    nc.vector.tensor_scalar_mul(
                        o_sbq[:, ii, :], otr[:, ii, :], lrecq[:, ii : ii + 1]
                    )
                nc.sync.dma_start(
                    out=out[s].rearrange("(b p t) d -> p b t d", p=128, t=QTB)[
                        :, qb, :, :
                    ],
                    in_=o_sbq,
                )

            pending = None
            pending_qb = None
            order = sorted(range(SLOTS), key=lambda x: -Ks[x])
            for idx, s in enumerate(order):
                Kv = Ks[s]
                is_last = idx == SLOTS - 1
                if s in preload:
                    kn, vn = preload.pop(s)
                else:
                    kn, vn = emit_kv_loads(s)
                # k -> kT [D, k]
                kTt = iop.tile([128, KM, 128], F32R, tag="kT")
                for g in range((Kv + 3) // 4):
                    n = min(4, Kv - g * 4)
                    trp = pstr.tile([128, 4, 128], F32R, tag="tr")
                    for ii in range(n):
                        nc.tensor.transpose(trp[:, ii, :], kn[:, g * 4 + ii, :], ident_r)
                    nc.scalar.copy(kTt[:, g * 4 : g * 4 + n, :], trp[:, :n, :])

                oT_slot = workp.tile([128, QT, 128], F32, tag="oT_slot", bufs=2)
                l_slot = workp.tile([1, L], F32, tag="l_slot")

                for qb in range(QB):
                    # lazy q load + transpose for this q block
                    qn = workp.tile([128, QTB, 128], F32R, tag="qn")
                    nc.sync.dma_start(
                        out=qn,
                        in_=q[s, qb * QBW : (qb + 1) * QBW, :].rearrange(
                            "(p t) d -> p t d", p=128
                        ),
                    )
                    qTt = workp.tile([128, QTB, 128], F32R, tag="qT")
                    trp = pstr.tile([128, 4, 128], F32R, tag="tr")
                    for ii in range(QTB):
                        nc.tensor.transpose(trp[:, ii, :], qn[:, ii, :], ident_r)
                    nc.vector.tensor_copy(qTt, trp)

                    oT_ps = psac.tile([128, QBW], F32, tag="oT")
                    l_ps = psac.tile([1, QBW], F32, tag="l")
                    for (j0, npair) in jgroups(Kv):
                        st = psst.tile([128, 2, QBW], F32, tag="st")
                        for jj in range(npair):
                            nc.tensor.matmul(
                                st[:, jj, :],
                                kTt[:, j0 + jj, :],
                                qTt,
                                start=True,
                                stop=True,
                            )
                        pT = workp.tile([128, 2, QBW], F32R, tag="pT")
                        last = j0 + npair == Kv
                        nc.scalar.activation(
                            pT[:, :npair, :],
                            st[:, :npair, :],
                            EXPF,
                            bias=(bias_sb[:, s : s + 1] if last else 0.0),
                            scale=INV_SQRT_D,
                        )
                        for jj in range(npair):
                            j = j0 + jj
                            nc.tensor.matmul(
                                oT_ps,
                                vn[:, j, :],
                                pT[:, jj, :],
                                start=(j == 0),
                                stop=(j == Kv - 1),
                            )
                            nc.tensor.matmul(
                                l_ps,
                                ones_r,
                                pT[:, jj, :],
                                start=(j == 0),
                                stop=(j == Kv - 1),
                            )
                    nc.vector.tensor_copy(oT_slot[:, qb * QTB : (qb + 1) * QTB, :], oT_ps)
                    if not is_last:
                        nc.scalar.copy(l_slot[:, qb * QBW : (qb + 1) * QBW], l_ps)
                    else:
                        l_sbq = workp.tile([1, QBW], F32, tag="l_sbq")
                        nc.scalar.copy(l_sbq, l_ps)
                        if pending_qb is not None:
                            emit_qb_finish(*pending_qb)
                        pending_qb = (s, qb, oT_slot, l_sbq)
                        if qb == 0 and pending is not None:
                            emit_finish(*pending)
                            pending = None

                if not is_last:
                    lrec = workp.tile([128, QT], F32, tag="lrec")
                    # l: [1, 2048] -> DRAM bounce -> [q%128, q//128]
                    lrows = dramp.tile([1, L], F32, tag="lrows")
                    nc.sync.dma_start(out=lrows, in_=l_slot)
                    lcol = workp.tile([128, QT], F32, tag="lcol")
                    nc.sync.dma_start(
                        out=lcol,
                        in_=lrows[0, :].rearrange("(t p) -> p t", p=128),
                    )
                    nc.vector.reciprocal(lrec, lcol)
                    # defer the O^T -> O finish by one slot so its l-latency
                    # hides under the next slot's compute
                    if pending is not None:
                        emit_finish(*pending)
                    pending = (s, oT_slot, lrec)
            if pending is not None:
                emit_finish(*pending)
            if pending_qb is not None:
                emit_qb_finish(*pending_qb)
    nc.compile()
    return nc


def _get_program(K0: int, K1: int):
    key = (K0, K1)
    if key not in _cache:
        _cache[key] = _build(K0, K1)
    return _cache[key]


def _run(q, k, v, valid_lens, trace=False):
    q = np.ascontiguousarray(np.asarray(q, dtype=np.float32))
    k = np.ascontiguousarray(np.asarray(k, dtype=np.float32))
    v = np.ascontiguousarray(np.asarray(v, dtype=np.float32))
    vl = np.asarray(valid_lens).astype(np.int64)
    K0 = int(max(1, -(-vl[0] // 128)))
    K1 = int(max(1, -(-vl[1] // 128)))
    KM = max(K0, K1)
    nc = _get_program(K0, K1)

    # per-slot mask bias column: 0 for valid positions in the last key tile,
    # -1e9 beyond valid_len
    biases = np.zeros((128, SLOTS), dtype=np.float32)
    Ks = [K0, K0, K1, K1]
    bs = [0, 0, 1, 1]
    pos = np.arange(128)
    for s in range(SLOTS):
        rem = int(vl[bs[s]]) - (Ks[s] - 1) * 128
        biases[:, s] = np.where(pos < rem, 0.0, np.float32(NEG))

    identf = np.eye(128, dtype=np.float32)
    onesr = np.ones((128, 1), dtype=np.float32)

    in_maps = []
    for c in range(NCORES):
        h0, h1 = 2 * c, 2 * c + 1
        qs = np.ascontiguousarray(
            np.stack([q[0, h0], q[0, h1], q[1, h0], q[1, h1]])
        )
        ks = np.ascontiguousarray(
            np.stack(
                [
                    k[0, h0, : KM * 128],
                    k[0, h1, : KM * 128],
                    k[1, h0, : KM * 128],
                    k[1, h1, : KM * 128],
                ]
            )
        )
        vs = np.ascontiguousarray(
            np.stack(
                [
                    v[0, h0, : KM * 128],
                    v[0, h1, : KM * 128],
                    v[1, h0, : KM * 128],
                    v[1, h1, : KM * 128],
                ]
            )
        )
        in_maps.append(
            {
                "q": qs,
                "k": ks,
                "v": vs,
                "identr": identf,
                "identf": identf,
                "onesr": onesr,
                "onef": onesr[:1, :1],
                "biases": biases,
            }
        )

    res = run_bass_kernel_spmd(
        nc, in_maps, core_ids=list(range(NCORES)), trace=trace
    )

    outp = np.empty((B, H, L, D), dtype=np.float32)
    for c in range(NCORES):
        o = res.results[c]["out"]
        h0, h1 = 2 * c, 2 * c + 1
        outp[0, h0] = o[0]
        outp[0, h1] = o[1]
        outp[1, h0] = o[2]
        outp[1, h1] = o[3]
    return outp, res


def kernel(q, k, v, valid_lens):
    outp, _ = _run(q, k, v, valid_lens, trace=False)
    return outp


# revision 20
# speedup vs baseline: 1.0463x; 1.0186x over previous
"""Masked dot-product attention (B=2,H=16,L=2048,D=128) on 8 trn2 NeuronCores.

Strategy:
  - Shard batch*heads: core c handles (b=0,h=2c),(0,2c+1),(1,2c),(1,2c+1) -> 4 slots.
  - Per (b,h): compute S^T[k,q] = K Q^T directly on the PE (lhsT = k-tile
    transposed to [D,k], rhs = q transposed to [D,q]) so softmax masking is a
    per-partition bias on the exp eviction, and no P-transposes are needed.
  - Only ceil(valid_len/128) key tiles are computed (the rest contribute
    exactly 0 after exp of -1e9, matching the reference's mask fill).
  - exp is fused into the PSUM->SBUF eviction on the scalar engine with
    scale = 1/sqrt(D); j pairs share one exp instruction. The last (partial)
    key tile gets a per-partition -1e9 bias.
  - O^T[d,q] += V_j^T P^T_j accumulates in PSUM (fp32); the softmax
    denominator l accumulates via all-ones [128,1] lhsT matmuls whose [1,512]
    outputs stack at partition offsets 32*qb inside a single PSUM bank.
  - l is reshaped to [q%128, q//128] layout via a DRAM bounce, reciprocal on
    the DVE, then O^T transposes back to [q,d] on the PE with the final
    eviction scaled by 1/l per partition. Hot matmuls use float32r (~12-bit
    mantissa, 4x fp32 PE throughput); accumulation stays fp32 in PSUM.
"""

import math
import os

import numpy as np

try:
    import concourse.bass as bass
except ImportError:  # pragma: no cover
    import sys

    sys.path.append("/opt/trn_rl_repo")
    import concourse.bass as bass

import concourse.mybir as mybir
import concourse.tile as tile
from concourse import bacc
from concourse.bass_utils import run_bass_kernel_spmd

B, H, L, D = 2, 16, 2048, 128
NCORES = 8
HPC = H // NCORES  # heads per core per batch
SLOTS = B * HPC  # bh slots per core
NEG = -1e9
INV_SQRT_D = 1.0 / math.sqrt(D)
F32 = mybir.dt.float32
F32R = mybir.dt.float32r
QT = L // 128  # 16 q tiles
QB = 4  # q blocks
QBW = L // QB  # 512 q per block
QTB = QT // QB  # 4 q tiles per block
EXPF = mybir.ActivationFunctionType.Exp

_cache: dict = {}


def _build(K0: int, K1: int):
    """Build+compile the per-core program for K0/K1 valid key tiles."""
    Ks = [K0, K0, K1, K1]
    KM = max(K0, K1)
    nc = bacc.Bacc("TRN2", target_bir_lowering=False, debug=False, num_devices=NCORES)
    q = nc.dram_tensor("q", [SLOTS, L, D], F32R, kind="ExternalInput")
    k = nc.dram_tensor("k", [SLOTS, KM * 128, D], F32R, kind="ExternalInput")
    v = nc.dram_tensor("v", [SLOTS, KM * 128, D], F32R, kind="ExternalInput")
    identr = nc.dram_tensor("identr", [128, 128], F32R, kind="ExternalInput")
    identf = nc.dram_tensor("identf", [128, 128], F32, kind="ExternalInput")
    onesr = nc.dram_tensor("onesr", [128, 1], F32R, kind="ExternalInput")
    onef = nc.dram_tensor("onef", [1, 1], F32, kind="ExternalInput")
    biases = nc.dram_tensor("biases", [128, SLOTS], F32, kind="ExternalInput")
    out = nc.dram_tensor("out", [SLOTS, L, D], F32, kind="ExternalOutput")

    # j pairs: all-but-last j grouped in twos, last j always alone (it takes
    # the mask bias)
    def jgroups(Kv):
        gs = []
        jj = 0
        while jj < Kv - 1:
            n = 2 if jj + 2 <= Kv - 1 else 1
            gs.append((jj, n))
            jj += n
        gs.append((Kv - 1, 1))
        return gs

    with tile.TileContext(nc) as tc:
        with (
            tc.tile_pool(name="const", bufs=1) as constp,
            tc.tile_pool(name="io", bufs=2) as iop,
            tc.tile_pool(name="work", bufs=3) as workp,
            tc.tile_pool(name="psst", bufs=2, space="PSUM") as psst,
            tc.tile_pool(name="pstr", bufs=2, space="PSUM") as pstr,
            tc.tile_pool(name="psac", bufs=1, space="PSUM") as psac,
            tc.tile_pool(name="dram", bufs=2, space="DRAM") as dramp,
        ):
            ident_r = constp.tile([128, 128], F32R)
            nc.sync.dma_start(out=ident_r, in_=identr[:, :])

            def emit_kv_loads(s):
                Kv = Ks[s]
                kn = iop.tile([128, KM, 128], F32R, tag="kn")
                nc.sync.dma_start(
                    out=kn[:, :Kv, :],
                    in_=k[s, : Kv * 128, :].rearrange("(t p) d -> p t d", p=128),
                )
                vn = iop.tile([128, KM, 128], F32R, tag="vn")
                nc.sync.dma_start(
                    out=vn[:, :Kv, :],
                    in_=v[s, : Kv * 128, :].rearrange("(t p) d -> p t d", p=128),
                )
                return kn, vn

            order0 = sorted(range(SLOTS), key=lambda x: -Ks[x])
            preload = {order0[0]: emit_kv_loads(order0[0])}
            qn0 = workp.tile([128, QTB, 128], F32R, tag="qn")
            nc.sync.dma_start(
                out=qn0,
                in_=q[order0[0], :QBW, :].rearrange("(p t) d -> p t d", p=128),
            )
            qn_preload = {(order0[0], 0): qn0}

            ident_f = constp.tile([128, 128], F32)
            nc.sync.dma_start(out=ident_f, in_=identf[:, :])
            ones_r = constp.tile([128, 1], F32R)
            nc.sync.dma_start(out=ones_r, in_=onesr[:, :])
            one_f = constp.tile([1, 1], F32)
            nc.sync.dma_start(out=one_f, in_=onef[:, :])
            bias_sb = constp.tile([128, SLOTS], F32)
            nc.sync.dma_start(out=bias_sb, in_=biases[:, :])

            def emit_finish(s, oT_slot, lrec):
                o_sb = workp.tile([128, QT, 128], F32, tag="o_sb", bufs=2)
                for g in range(QT // 4):
                    otr = pstr.tile([128, 4, 128], F32, tag="tr")
                    for ii in range(4):
                        nc.tensor.transpose(
                            otr[:, ii, :], oT_slot[:, g * 4 + ii, :], ident_f
                        )
                    for ii in range(4):
                        t = g * 4 + ii
                        nc.vector.tensor_scalar_mul(
                            o_sb[:, t, :], otr[:, ii, :], lrec[:, t : t + 1]
                        )
                nc.sync.dma_start(
                    out=out[s].rearrange("(b p t) d -> p b t d", p=128, t=QTB),
                    in_=o_sb.rearrange("p (b t) d -> p b t d", t=QTB),
                )

            def emit_qb_finish(s, qb, oT_slot, l_sbq):
                # tiny K=1 matmuls transpose l into per-partition layout
                ltq = pstr.tile([128, QTB], F32, tag="tr")
                for t in range(QTB):
                    nc.tensor.matmul(
                        ltq[:, t : t + 1],
                        l_sbq[:, t * 128 : (t + 1) * 128],
                        one_f[:, :],
                        start=(t == 0),
                        stop=(t == QTB - 1),
                        skip_group_check=True,
                    )
                lrecq = workp.tile([128, QTB], F32, tag="lrecq")
                nc.vector.reciprocal(lrecq, ltq)
                o_sbq = workp.tile([128, QTB, 128], F32, tag="o_sbq")
                otr = pstr.tile([128, 4, 128], F32, tag="tr")
                for ii in range(QTB):
                    nc.tensor.transpose(
                        otr[:, ii, :], oT_slot[:, qb * QTB + ii, :], ident_f
                    )
                for ii in range(QTB):
                    nc.vector.tensor_scalar_mul(
                        o_sbq[:, ii, :], otr[:, ii, :], lrecq[:, ii : ii + 1]
                    )
                nc.sync.dma_start(
                    out=out[s].rearrange("(b p t) d -> p b t d", p=128, t=QTB)[
                        :, qb, :, :
                    ],
                    in_=o_sbq,
                )

            pending = None
            pending_qb = None
            order = sorted(range(SLOTS), key=lambda x: -Ks[x])
            for idx, s in enumerate(order):
                Kv = Ks[s]
                is_last = idx == SLOTS - 1
                if s in preload:
                    kn, vn = preload.pop(s)
                else:
                    kn, vn = emit_kv_loads(s)
                # k -> kT [D, k]
                kTt = iop.tile([128, KM, 128], F32R, tag="kT")
                for g in range((Kv + 3) // 4):
                    n = min(4, Kv - g * 4)
                    trp = pstr.tile([128, 4, 128], F32R, tag="tr")
                    for ii in range(n):
                        nc.tensor.transpose(trp[:, ii, :], kn[:, g * 4 + ii, :], ident_r)
                    nc.scalar.copy(kTt[:, g * 4 : g * 4 + n, :], trp[:, :n, :])

                oT_slot = workp.tile([128, QT, 128], F32, tag="oT_slot", bufs=2)
                l_slot = workp.tile([1, L], F32, tag="l_slot")

                for qb in range(QB):
                    # lazy q load + transpose for this q block
                    if (s, qb) in qn_preload:
                        qn = qn_preload.pop((s, qb))
                    else:
                        qn = workp.tile([128, QTB, 128], F32R, tag="qn")
                        nc.sync.dma_start(
                            out=qn,
                            in_=q[s, qb * QBW : (qb + 1) * QBW, :].rearrange(
                                "(p t) d -> p t d", p=128
                            ),
                        )
                    qTt = workp.tile([128, QTB, 128], F32R, tag="qT")
                    trp = pstr.tile([128, 4, 128], F32R, tag="tr")
                    for ii in range(QTB):
                        nc.tensor.transpose(trp[:, ii, :], qn[:, ii, :], ident_r)
                    nc.vector.tensor_copy(qTt, trp)

                    oT_ps = psac.tile([128, QBW], F32, tag="oT")
                    l_ps = psac.tile([1, QBW], F32, tag="l")
                    for (j0, npair) in jgroups(Kv):
                        st = psst.tile([128, 2, QBW], F32, tag="st")
                        for jj in range(npair):
                            nc.tensor.matmul(
                                st[:, jj, :],
                                kTt[:, j0 + jj, :],
                                qTt,
                                start=True,
                                stop=True,
                            )
                        pT = workp.tile([128, 2, QBW], F32R, tag="pT")
                        last = j0 + npair == Kv
                        nc.scalar.activation(
                            pT[:, :npair, :],
                            st[:, :npair, :],
                            EXPF,
                            bias=(bias_sb[:, s : s + 1] if last else 0.0),
                            scale=INV_SQRT_D,
                        )
                        for jj in range(npair):
                            j = j0 + jj
                            nc.tensor.matmul(
                                oT_ps,
                                vn[:, j, :],
                                pT[:, jj, :],
                                start=(j == 0),
                                stop=(j == Kv - 1),
                            )
                            nc.tensor.matmul(
                                l_ps,
                                ones_r,
                                pT[:, jj, :],
                                start=(j == 0),
                                stop=(j == Kv - 1),
                            )
                    nc.vector.tensor_copy(oT_slot[:, qb * QTB : (qb + 1) * QTB, :], oT_ps)
                    if not is_last:
                        nc.scalar.copy(l_slot[:, qb * QBW : (qb + 1) * QBW], l_ps)
                    else:
                        l_sbq = workp.tile([1, QBW], F32, tag="l_sbq")
                        nc.scalar.copy(l_sbq, l_ps)
                        if pending_qb is not None:
                            emit_qb_finish(*pending_qb)
                        pending_qb = (s, qb, oT_slot, l_sbq)
                        if qb == 0 and pending is not None:
                            emit_finish(*pending)
                            pending = None

                if not is_last:
                    lrec = workp.tile([128, QT], F32, tag="lrec")
                    # l: [1, 2048] -> DRAM bounce -> [q%128, q//128]
                    lrows = dramp.tile([1, L], F32, tag="lrows")
                    nc.sync.dma_start(out=lrows, in_=l_slot)
                    lcol = workp.tile([128, QT], F32, tag="lcol")
                    nc.sync.dma_start(
                        out=lcol,
                        in_=lrows[0, :].rearrange("(t p) -> p t", p=128),
                    )
                    nc.vector.reciprocal(lrec, lcol)
                    # defer the O^T -> O finish by one slot so its l-latency
                    # hides under the next slot's compute
                    if pending is not None:
                        emit_finish(*pending)
                    pending = (s, oT_slot, lrec)
            if pending is not None:
                emit_finish(*pending)
            if pending_qb is not None:
                emit_qb_finish(*pending_qb)
    nc.compile()
    return nc


def _get_program(K0: int, K1: int):
    key = (K0, K1)
    if key not in _cache:
        _cache[key] = _build(K0, K1)
    return _cache[key]


def _run(q, k, v, valid_lens, trace=False):
    q = np.ascontiguousarray(np.asarray(q, dtype=np.float32))
    k = np.ascontiguousarray(np.asarray(k, dtype=np.float32))
    v = np.ascontiguousarray(np.asarray(v, dtype=np.float32))
    vl = np.asarray(valid_lens).astype(np.int64)
    K0 = int(max(1, -(-vl[0] // 128)))
    K1 = int(max(1, -(-vl[1] // 128)))
    KM = max(K0, K1)
    nc = _get_program(K0, K1)

    # per-slot mask bias column: 0 for valid positions in the last key tile,
    # -1e9 beyond valid_len
    biases = np.zeros((128, SLOTS), dtype=np.float32)
    Ks = [K0, K0, K1, K1]
    bs = [0, 0, 1, 1]
    pos = np.arange(128)
    for s in range(SLOTS):
        rem = int(vl[bs[s]]) - (Ks[s] - 1) * 128
        biases[:, s] = np.where(pos < rem, 0.0, np.float32(NEG))

    identf = np.eye(128, dtype=np.float32)
    onesr = np.ones((128, 1), dtype=np.float32)

    in_maps = []
    for c in range(NCORES):
        h0, h1 = 2 * c, 2 * c + 1
        qs = np.ascontiguousarray(
            np.stack([q[0, h0], q[0, h1], q[1, h0], q[1, h1]])
        )
        ks = np.ascontiguousarray(
            np.stack(
                [
                    k[0, h0, : KM * 128],
                    k[0, h1, : KM * 128],
                    k[1, h0, : KM * 128],
                    k[1, h1, : KM * 128],
                ]
            )
        )
        vs = np.ascontiguousarray(
            np.stack(
                [
                    v[0, h0, : KM * 128],
                    v[0, h1, : KM * 128],
                    v[1, h0, : KM * 128],
                    v[1, h1, : KM * 128],
                ]
            )
        )
        in_maps.append(
            {
                "q": qs,
                "k": ks,
                "v": vs,
                "identr": identf,
                "identf": identf,
                "onesr": onesr,
                "onef": onesr[:1, :1],
                "biases": biases,
            }
        )

    res = run_bass_kernel_spmd(
        nc, in_maps, core_ids=list(range(NCORES)), trace=trace
    )

    outp = np.empty((B, H, L, D), dtype=np.float32)
    for c in range(NCORES):
        o = res.results[c]["out"]
        h0, h1 = 2 * c, 2 * c + 1
        outp[0, h0] = o[0]
        outp[0, h1] = o[1]
        outp[1, h0] = o[2]
        outp[1, h1] = o[3]
    return outp, res


def kernel(q, k, v, valid_lens):
    outp, _ = _run(q, k, v, valid_lens, trace=False)
    return outp
